# revision 2
# baseline (speedup 1.0000x reference)
import sys

sys.path.insert(0, "/opt/trn_rl_repo")

import numpy as np

import concourse.bacc as bacc
import concourse.bass as bass
import concourse.mybir as mybir
import concourse.tile as tile
from concourse.bass_utils import run_bass_kernel_spmd

F32 = mybir.dt.float32

N, M, G, A, H = 20000, 48, 16, 64, 16
NCORES = 8
NL = N // NCORES  # 2500 atoms per core
CW = 160  # acat cols: [0:64]=a, [64:80]=gs, [80:160]=gv d-major 32-padded
NB2 = 256  # stage-2 group size (atoms)
NB1 = 8  # stage-1 subgroup size: 4 "even" (parts 0-47) + 4 "odd" (64-111)

_nc_cache = {}


def _dummy_mm(nc, out_ap, src_ap, tp):
    # K=1/M=1/N=1 matmul whose only job is to absorb one semaphore wait
    # (this walrus encodes at most one sync-wait per PE instruction).
    nc.tensor.matmul(
        out=out_ap, lhsT=src_ap, rhs=src_ap, start=True, stop=True, tile_position=tp
    )


def _build(nl=NL, nb2=NB2, sim=False):
    """Per-core Bass program.

    Stage 1 (per atom): psum1[32d+g, a] = sum_m gv[n,m,g,d]*a[n,m,a]
      (lhsT = acat gv-block [48m, 80], rhs = acat a-block [48m, 64]);
      atom pairs packed on PE row-group bases 0/64.
    Stage 1b: psum_s[a, g] = sum_m a[n,m,a]*gs[n,m,g].
    Stage 2 (per 256-atom group, channel a, axis d): K=16 matmul of
      agh[a] against vbig rows 32d..32d+16, 3x4 tile_position packing.
    Finish: ACT square, DVE d-sum, DVE 32x32 block-transpose -> n on
      partitions, 64B-chunk DMA out.
    """
    nc = bacc.Bacc("TRN2", target_bir_lowering=False)
    ac_d = nc.declare_dram_parameter("acat", [nl, M, CW], F32, isOutput=False)
    w_d = nc.declare_dram_parameter("aghw", [96, A * H], F32, isOutput=False)
    out_d = nc.declare_dram_parameter("out", [nl, A * G + A * H], F32, isOutput=True)

    Sq = mybir.ActivationFunctionType.Square

    with tile.TileContext(nc) as tc:
        with (
            tc.tile_pool(name="singles", bufs=1) as singles,
            tc.tile_pool(name="ain", bufs=4) as ain_pool,
            tc.tile_pool(name="vbig", bufs=1) as vbig_pool,
            tc.tile_pool(name="ssb", bufs=2) as ssb_pool,
            tc.tile_pool(name="sq", bufs=2) as sq_pool,
            tc.tile_pool(name="ov", bufs=2) as ov_pool,
            tc.tile_pool(name="ovt", bufs=2) as ovt_pool,
            tc.tile_pool(name="psum1", bufs=2, space="PSUM") as p1_pool,
            tc.tile_pool(name="psums", bufs=2, space="PSUM") as ps_pool,
            tc.tile_pool(name="psum2", bufs=1, space="PSUM") as p2_pool,
        ):
            aghw = singles.tile([96, A * H], F32)
            nc.sync.dma_start(out=aghw[:, :], in_=w_d[:, :])

            ngroups = (nl + nb2 - 1) // nb2
            for g2 in range(ngroups):
                n0 = g2 * nb2
                ncnt = min(nb2, nl - n0)
                vbig = vbig_pool.tile([80, nb2 * A], F32)

                # ---- stage 1 ----
                nflush = (ncnt + 31) // 32
                for fl in range(nflush):
                    fn0 = n0 + fl * 32
                    fcnt = min(32, n0 + ncnt - fn0)
                    psum_s = ps_pool.tile([128, 512], F32)
                    nsub = (fcnt + NB1 - 1) // NB1
                    first_sub = True
                    for sub in range(nsub):
                        sn0 = fn0 + sub * NB1
                        scnt = min(NB1, fn0 + fcnt - sn0)
                        acs = ain_pool.tile([128, NB1 * CW], F32)
                        nc.sync.dma_start(
                            out=acs[0:M, 0 : scnt * CW].rearrange(
                                "p (n c) -> p n c", c=CW
                            ),
                            in_=ac_d[sn0 : sn0 + scnt].rearrange("n m c -> m n c"),
                        )
                        psum1 = p1_pool.tile([128, NB1 * A], F32)
                        # wait absorbers (<=1 sync-wait per PE instruction):
                        # psum_s WAR (ACT), psum1 WAR (DVE), input DMA
                        if first_sub:
                            _dummy_mm(nc, psum_s[96:97, 0:1], aghw[0:1, 0:1], (0, 96))
                            first_sub = False
                        _dummy_mm(nc, psum1[96:97, 0:1], aghw[0:1, 0:1], (0, 96))
                        _dummy_mm(nc, psum1[96:97, 1:2], acs[0:1, 0:1], (0, 96))
                        for j in range(scnt):
                            c0 = j * CW
                            nc.tensor.matmul(
                                out=psum1[0:80, j * A : (j + 1) * A],
                                lhsT=acs[0:M, c0 + 80 : c0 + CW],
                                rhs=acs[0:M, c0 : c0 + A],
                                start=True,
                                stop=True,
                            )
                            je = sub * NB1 + j
                            nc.tensor.matmul(
                                out=psum_s[0:A, je * G : (je + 1) * G],
                                lhsT=acs[0:M, c0 : c0 + A],
                                rhs=acs[0:M, c0 + A : c0 + A + G],
                                start=True,
                                stop=True,
                            )
                        nc.vector.tensor_copy(
                            out=vbig[
                                :,
                                (fl * 32 + sub * NB1)
                                * A : (fl * 32 + sub * NB1 + scnt)
                                * A,
                            ],
                            in_=psum1[0:80, 0 : scnt * A],
                        )
                    s_sb = ssb_pool.tile([128, 512], F32)
                    nc.scalar.copy(
                        out=s_sb[0:A, 0 : fcnt * G], in_=psum_s[0:A, 0 : fcnt * G]
                    )
                    dst = out_d[fn0 : fn0 + fcnt, 0 : A * G].rearrange(
                        "n (a g) -> a n g", g=G
                    )
                    nc.sync.dma_start(
                        out=dst,
                        in_=s_sb[0:A, 0 : fcnt * G].rearrange(
                            "p (n g) -> p n g", g=G
                        ),
                    )

                # ---- stage 2: psum2 d-slices in separate banks ----
                rhs_full = vbig[:, :].rearrange("p (n a) -> p n a", a=A)
                for q in range(16):
                    psum2 = p2_pool.tile([128, 1536], F32)
                    _dummy_mm(nc, psum2[0:1, 300:301], aghw[0:1, 0:1], (0, 0))
                    for c in range(4):
                        a_ch = q * 4 + c
                        for d in range(3):
                            nc.tensor.matmul(
                                out=psum2[
                                    32 * c : 32 * c + H, d * 512 : d * 512 + ncnt
                                ],
                                lhsT=aghw[
                                    32 * d : 32 * d + G, a_ch * H : (a_ch + 1) * H
                                ],
                                rhs=rhs_full[
                                    32 * d : 32 * d + G, 0:ncnt, a_ch : a_ch + 1
                                ],
                                start=True,
                                stop=True,
                                tile_position=(32 * d, 32 * c),
                            )
                    if sim:
                        for r0 in (16, 48, 80):
                            for d in range(3):
                                nc.vector.memset(
                                    psum2[r0 : r0 + 16, d * 512 : d * 512 + 256], 0.0
                                )
                        if ncnt < 256:
                            for d in range(3):
                                nc.vector.memset(
                                    psum2[0:112, d * 512 + ncnt : d * 512 + 256], 0.0
                                )
                    sq = sq_pool.tile([128, 768], F32)
                    nc.scalar.activation(
                        out=sq[0:112, :],
                        in_=psum2[0:112, :].rearrange(
                            "p (d z) -> p d z", z=512
                        )[:, :, 0:256],
                        func=Sq,
                    )
                    ov = ov_pool.tile([128, 256], F32)
                    if sim:
                        nc.vector.memset(ov[96:128, 0:256], 0.0)
                    nc.vector.tensor_add(
                        ov[0:112, 0:ncnt],
                        sq[0:112, 0:ncnt],
                        sq[0:112, 256 : 256 + ncnt],
                    )
                    nc.vector.tensor_add(
                        ov[0:112, 0:ncnt],
                        ov[0:112, 0:ncnt],
                        sq[0:112, 512 : 512 + ncnt],
                    )
                    if ncnt < 256:
                        nc.vector.memset(ov[0:128, ncnt:256], 0.0)
                    ovt = ovt_pool.tile([128, 256], F32)
                    nc.vector.transpose(out=ovt[:, :], in_=ov[:, :])
                    # ovt[32c + n%32, 32*(n//32) + h] = out_v[n0+n, 4q+c, h]
                    nbfull = ncnt // 32
                    nrem = ncnt - nbfull * 32
                    for c in range(4):
                        a_ch = q * 4 + c
                        col0 = A * G + a_ch * H
                        if nbfull:
                            src = ovt[32 * c : 32 * c + 32, :].rearrange(
                                "p (b h2) -> p b h2", h2=32
                            )[:, 0:nbfull, 0:H]
                            dst = out_d[
                                n0 : n0 + nbfull * 32, col0 : col0 + H
                            ].rearrange("(b x) h -> x b h", x=32)
                            nc.sync.dma_start(out=dst, in_=src)
                        if nrem:
                            src = ovt[
                                32 * c : 32 * c + nrem,
                                nbfull * 32 : nbfull * 32 + H,
                            ]
                            dst = out_d[
                                n0 + nbfull * 32 : n0 + ncnt, col0 : col0 + H
                            ]
                            nc.sync.dma_start(out=dst, in_=src)
    nc.compile()
    return nc


def _get_nc():
    if "nc" not in _nc_cache:
        _nc_cache["nc"] = _build()
    return _nc_cache["nc"]


def _prep(a, gs, gv, agh):
    acat = np.zeros((N, M, CW), np.float32)
    acat[:, :, 0:A] = a
    acat[:, :, A : A + G] = gs
    for d in range(3):
        acat[:, :, 80 + 32 * d : 96 + 32 * d] = gv[:, :, :, d]
    aghw = np.zeros((96, A * H), np.float32)
    base = np.ascontiguousarray(np.asarray(agh, np.float32).transpose(1, 0, 2)).reshape(
        G, A * H
    )
    for r in range(3):
        aghw[32 * r : 32 * r + G] = base
    return acat, aghw


def _in_maps(inputs):
    a = np.asarray(inputs["a"], np.float32)
    gs = np.asarray(inputs["gs"], np.float32)
    gv = np.asarray(inputs["gv"], np.float32)
    acat, aghw = _prep(a, gs, gv, inputs["agh"])
    return [
        {"acat": acat[c * NL : (c + 1) * NL], "aghw": aghw} for c in range(NCORES)
    ]


def kernel(a, gs, gv, agh):
    nc = _get_nc()
    in_maps = _in_maps({"a": a, "gs": gs, "gv": gv, "agh": agh})
    res = run_bass_kernel_spmd(nc, in_maps, list(range(NCORES))).results
    return np.concatenate([res[c]["out"] for c in range(NCORES)], axis=0)



# revision 26
# speedup vs baseline: 2.4622x; 2.4622x over previous
import os
import sys

sys.path.insert(0, "/opt/trn_rl_repo")

DBG = int(os.environ.get("KDBG", "3"))  # 1=stage1, 2=+stage2+finish1, 3=full

import numpy as np
import ml_dtypes

import concourse.bacc as bacc
import concourse.bass as bass
import concourse.mybir as mybir
import concourse.tile as tile
from concourse.bass_utils import run_bass_kernel_spmd

F32 = mybir.dt.float32
BF16 = mybir.dt.bfloat16
BF = ml_dtypes.bfloat16

N, M, G, A, H = 20000, 48, 16, 64, 16
NCORES = 8
NL = N // NCORES      # 2500 atoms per core
NPAIR = NL // 2       # 1250 atom pairs
GJ = 128              # pairs per group (256 atoms)
TP = 8                # pairs per psum1 tile / lhsT tile

_nc_cache = {}


def _build():
    """Per-core Bass program, all-bf16 PE pipeline.

    Stage 1 (per atom-pair, K=96 = 2 atoms x 48 m):
      lhsT [96,128]: rows 0:48 even atom's [gv(d0,d1,d2)|gs] in cols 0:64
      (zeros in 48:96), rows 48:96 odd atom's in cols 64:128 (zeros in 0:48).
      rhs [96,64] = both atoms' a stacked. One matmul -> psum1[0:128, 64 cols]:
      rows = (parity, dgs-slot, g): e:d0@0,d1@16,d2@32,S'@48; o:+64.
    Copy psum1 -> vbig bf16 [128, (a=64, j)]
    Stage 2 (per q-tile of 4 channels): for (slot s, parity p, ch-pair):
      wide lhsT [16,32]=(0|agh[odd]) start=T then narrow [16,16]=agh[even]
      start=F at tile_position (32*((64p+16s)//32), 64p+32cp).
      psum2 [128, 512] rows (p, c, h) cols (slot, j).  Slot 3 uses identity
      weights -> out_s passthrough.
    Finish: ACT square d-slots -> sq bf16; DVE 2 adds -> ovin[:,0:128];
      ACT copy S'-slot -> ovin[:,128:256]; PE transpose x2 -> psum_t
      [j, (vs,p,c,h)]; copy -> SBUF; 2 DMAs out with 256B runs.
    """
    nc = bacc.Bacc("TRN2", target_bir_lowering=False)
    aw_d = nc.declare_dram_parameter("aw", [2, M, NPAIR, 64], BF16, isOutput=False)
    ar_d = nc.declare_dram_parameter("ar", [2, M, NPAIR, 64], BF16, isOutput=False)
    # stage-2 weights: per (slot s, channel): [128, 32] block, cols (p, h),
    # nonzero only at K-rows 64p+16s+g (baked zeros select slot+parity)
    wg_d = nc.declare_dram_parameter("wg", [128, 8192], BF16, isOutput=False)
    id_d = nc.declare_dram_parameter("ident", [128, 128], BF16, isOutput=False)
    out_d = nc.declare_dram_parameter("out", [NL, A * G + A * H], F32, isOutput=True)

    Sq = mybir.ActivationFunctionType.Square

    ngroups = (NPAIR + GJ - 1) // GJ  # 10

    with tile.TileContext(nc) as tc:
        with (
            tc.tile_pool(name="singles", bufs=1) as singles,
            tc.tile_pool(name="lw", bufs=3) as lw_pool,
            tc.tile_pool(name="ar", bufs=3) as ar_pool,
            tc.tile_pool(name="vbig", bufs=2) as vbig_pool,
            tc.tile_pool(name="sq", bufs=2) as sq_pool,
            tc.tile_pool(name="ovin", bufs=16) as ovin_pool,
            tc.tile_pool(name="ot", bufs=3) as ot_pool,
            tc.tile_pool(name="psum1", bufs=3, space="PSUM") as p1_pool,
            tc.tile_pool(name="psum2", bufs=2, space="PSUM") as p2_pool,
            tc.tile_pool(name="psumt", bufs=2, space="PSUM") as pt_pool,
        ):
            wg = singles.tile([128, 8192], BF16)
            nc.sync.dma_start(out=wg[:, :], in_=wg_d[:, :])
            ident = singles.tile([128, 128], BF16)
            nc.sync.dma_start(out=ident[:, :], in_=id_d[:, :])

            # persistent zero-padded lhsT/rhs rings (zeros must survive rotation)
            lws = [
                lw_pool.tile([128, TP * 128], BF16, name=f"lw{i}") for i in range(3)
            ]
            for lw in lws:
                nc.vector.memset(lw[0:128, :], 0.0)
            ars = [
                ar_pool.tile([128, TP * 64], BF16, name=f"arr{i}") for i in range(3)
            ]
            for arr in ars:
                nc.vector.memset(arr[96:128, :], 0.0)

            tglob = 0
            for g2 in range(ngroups):
                j0 = g2 * GJ
                jcnt = min(GJ, NPAIR - j0)  # pairs in group
                n0 = 2 * j0
                vbig = vbig_pool.tile([128, A * GJ], BF16)
                vb = vbig[:, :].rearrange("p (a j) -> p a j", j=GJ)

                # ---- stage 1 ----
                ntiles = (jcnt + TP - 1) // TP
                for t in range(ntiles):
                    tj0 = t * TP
                    tjc = min(TP, jcnt - tj0)
                    Tg = j0 + tj0  # global pair idx
                    lw = lws[tglob % 3]
                    arr = ars[tglob % 3]
                    tglob += 1
                    lwv = lw[:, :].rearrange("p (k z) -> p k z", z=128)
                    nc.sync.dma_start(
                        out=lwv[0:48, 0:tjc, 0:64],
                        in_=aw_d[0, :, Tg : Tg + tjc, :],
                    )
                    nc.sync.dma_start(
                        out=lwv[48:96, 0:tjc, 64:128],
                        in_=aw_d[1, :, Tg : Tg + tjc, :],
                    )
                    arv = arr[:, :].rearrange("p (k z) -> p k z", z=64)
                    nc.sync.dma_start(
                        out=arv[0:48, 0:tjc, :], in_=ar_d[0, :, Tg : Tg + tjc, :]
                    )
                    nc.sync.dma_start(
                        out=arv[48:96, 0:tjc, :], in_=ar_d[1, :, Tg : Tg + tjc, :]
                    )
                    psum1 = p1_pool.tile([128, 512], F32)
                    if DBG < 1:
                        continue
                    for k in range(tjc):
                        nc.tensor.matmul(
                            out=psum1[0:128, 64 * k : 64 * k + 64],
                            lhsT=lw[0:128, 128 * k : 128 * k + 128],
                            rhs=arr[0:128, 64 * k : 64 * k + 64],
                            start=True,
                            stop=True,
                        )
                    # psum -> vbig (bf16), alternate DVE/ACT
                    src = psum1[:, 0 : 64 * tjc].rearrange("p (k a) -> p a k", a=64)
                    dst = vb[:, :, tj0 : tj0 + tjc]
                    if t % 2 == 0:
                        nc.vector.tensor_copy(out=dst, in_=src)
                    else:
                        nc.scalar.copy(out=dst, in_=src)

                # ---- stage 2 + finish part 1 ----
                if DBG < 2:
                    # drain vbig to out so the program has output deps
                    dbgt = ot_pool.tile([128, 256], F32)
                    nc.vector.tensor_copy(out=dbgt[:, :], in_=vbig[:, 0:256])
                    nc.sync.dma_start(
                        out=out_d[n0 : n0 + 128, 0:256], in_=dbgt[:, :]
                    )
                    continue
                ovins = []
                for q in range(16):
                    psum2 = p2_pool.tile([128, 512], F32)
                    # Column-tiling only (128x32 tiles): K=128 covers all of
                    # psum1's rows (x, dgs, g); the baked-zero weights select
                    # (slot s, parity) per output row block.  psum2 rows
                    # (c, p, h) = 32c+16p+h, cols (s, j) = 128s+j (one bank).
                    for s in range(4):
                        for c in range(4):
                            ch = 4 * q + c
                            c0 = 32 * (64 * s + ch)
                            nc.tensor.matmul(
                                out=psum2[32 * c : 32 * c + 32,
                                          128 * s : 128 * s + jcnt],
                                lhsT=wg[0:128, c0 : c0 + 32],
                                rhs=vb[0:128, ch, 0:jcnt],
                                start=True,
                                stop=True,
                                tile_position=(0, 32 * c),
                            )
                    sq = sq_pool.tile([128, 384], BF16)
                    nc.scalar.activation(
                        out=sq[:, :],
                        in_=psum2[:, 0:384],
                        func=Sq,
                    )
                    ovin = ovin_pool.tile([128, 256], BF16)
                    ovins.append(ovin)
                    nc.vector.tensor_add(
                        ovin[:, 0:128], sq[:, 0:128], sq[:, 128:256]
                    )
                    nc.vector.tensor_add(
                        ovin[:, 0:128], ovin[:, 0:128], sq[:, 256:384]
                    )
                    nc.scalar.copy(out=ovin[:, 128:256], in_=psum2[:, 384:512])

                # ---- finish part 2: transposes + out DMA ----
                if DBG < 3:
                    for q in range(16):
                        ovin = ovins[q]
                        dbgt = ot_pool.tile([128, 256], F32)
                        nc.vector.tensor_copy(out=dbgt[:, :], in_=ovin[:, :])
                        nc.sync.dma_start(
                            out=out_d[n0 : n0 + 128, 256:512], in_=dbgt[:, :]
                        )
                    continue
                for q in range(16):
                    ovin = ovins[q]
                    psum_t = pt_pool.tile([128, 256], BF16)
                    nc.tensor.transpose(
                        out=psum_t[0:jcnt, 0:128],
                        in_=ovin[:, 0:jcnt],
                        identity=ident[:, :],
                    )
                    nc.tensor.transpose(
                        out=psum_t[0:jcnt, 128:256],
                        in_=ovin[:, 128 : 128 + jcnt],
                        identity=ident[:, :],
                    )
                    ot = ot_pool.tile([128, 256], F32)
                    if q % 2 == 0:
                        nc.vector.tensor_copy(
                            out=ot[0:jcnt, :], in_=psum_t[0:jcnt, :]
                        )
                    else:
                        nc.scalar.copy(out=ot[0:jcnt, :], in_=psum_t[0:jcnt, :])
                    # ot cols = (vs 2, c 4, p 2, h 16); per parity p one DMA
                    # with (c,h)-contiguous 256B HBM runs
                    otv = ot[0:jcnt, :].rearrange(
                        "j (v c p h) -> j v c p h", v=2, c=4, p=2
                    )
                    for vs in range(2):
                        colb = 1024 * (1 - vs) + 64 * q
                        dst4 = out_d[
                            n0 : n0 + 2 * jcnt, colb : colb + 64
                        ].rearrange("(j p) (c h) -> j p c h", p=2, h=16)
                        for p in range(2):
                            nc.sync.dma_start(
                                out=dst4[:, p], in_=otv[:, vs, :, p, :]
                            )
    nc.compile()
    return nc


def _get_nc():
    if "nc" not in _nc_cache:
        _nc_cache["nc"] = _build()
    return _nc_cache["nc"]


def _prep(a, gs, gv, agh):
    """Host-side packing into the per-core HBM layouts (bf16)."""
    a = np.asarray(a, np.float32)
    gs = np.asarray(gs, np.float32)
    gv = np.asarray(gv, np.float32)
    agh = np.asarray(agh, np.float32)

    # weights per atom: [gv d0 | gv d1 | gv d2 | gs] (16 each) -> 64 cols
    wcat = np.empty((N, M, 64), dtype=BF)
    for d in range(3):
        wcat[:, :, 16 * d : 16 * d + 16] = gv[:, :, :, d].astype(BF)
    wcat[:, :, 48:64] = gs.astype(BF)

    a16 = a.astype(BF)

    # split into per-core, even/odd pairs, m-major
    # aw[core][p, m, j, :] = wcat[n0 + 2j + p, m, :]
    aw = np.empty((NCORES, 2, M, NPAIR, 64), dtype=BF)
    ar = np.empty((NCORES, 2, M, NPAIR, 64), dtype=BF)
    wc = wcat.reshape(NCORES, NPAIR, 2, M, 64)
    ac = a16.reshape(NCORES, NPAIR, 2, M, 64)
    for p in range(2):
        aw[:, p] = wc[:, :, p].transpose(0, 2, 1, 3)
        ar[:, p] = ac[:, :, p].transpose(0, 2, 1, 3)

    # stage-2 weights: block (s, ch) = [128, 32], cols (p 2, h 16), value
    # agh[ch][g, h] (ident for s=3) at K-rows 64p+16s+g, zeros elsewhere
    wgm = np.zeros((128, 8192), dtype=BF)
    aghT = agh.transpose(1, 0, 2).astype(BF)  # [g, a, h]
    eye = np.eye(16, dtype=BF)
    for s in range(4):
        for ch in range(A):
            c0 = 32 * (64 * s + ch)
            blk = eye if s == 3 else aghT[:, ch, :]
            for p in range(2):
                r0 = 64 * p + 16 * s
                wgm[r0 : r0 + 16, c0 + 16 * p : c0 + 16 * p + 16] = blk

    ident = np.eye(128, dtype=BF)
    return aw, ar, wgm, ident


def _in_maps(inputs):
    aw, ar, wgm, ident = _prep(
        inputs["a"], inputs["gs"], inputs["gv"], inputs["agh"]
    )
    return [
        {"aw": aw[c], "ar": ar[c], "wg": wgm, "ident": ident}
        for c in range(NCORES)
    ]


def kernel(a, gs, gv, agh):
    nc = _get_nc()
    in_maps = _in_maps({"a": a, "gs": gs, "gv": gv, "agh": agh})
    res = run_bass_kernel_spmd(nc, in_maps, list(range(NCORES))).results
    return np.concatenate([res[c]["out"] for c in range(NCORES)], axis=0)


# revision 33
# speedup vs baseline: 7.4655x; 3.0321x over previous
import os
import sys

sys.path.insert(0, "/opt/trn_rl_repo")

import numpy as np
import ml_dtypes

import concourse.bacc as bacc
import concourse.bass as bass
import concourse.mybir as mybir
import concourse.tile as tile
from concourse.bass_utils import run_bass_kernel_spmd

F32 = mybir.dt.float32
BF16 = mybir.dt.bfloat16
BF = ml_dtypes.bfloat16

N, M, G, A, H = 20000, 48, 16, 64, 16
NCORES = 8
NL = N // NCORES      # 2500 atoms per core
NPAIR = NL // 2       # 1250 atom pairs
GJ = 256              # pairs per group (512 atoms)
KJ = 32               # pairs per input DMA chunk
TP = 8                # pairs per psum1 tile

_nc_cache = {}


def _build():
    """Per-core Bass program, bf16 PE pipeline, fp32 psum/output.

    Stage 1 (per atom-pair): one [128,128]x[128,64] matmul.  lhsT rows
    (x=parity 0:96, zero rows 96:128 for the K=128 FWL pad), cols
    (x', dgs, g) with parity-zeros baked in HBM.  psum1 rows
    (x, dgs, g): e: d0@0,d1@16,d2@32,S'@48; o: +64.  Copy -> vbig bf16.
    Stage 2 (q-tile = 4 channels x 256 pairs): 16 matmuls, K=128 full
    rows, lhsT [128,32] (cols (p,h), baked zeros select slot+parity),
    column tiles (0,32c).  psum2 rows (c,p,h), cols (slot, j).
    Finish: ACT square -> sq bf16; DVE adds (d-sum) -> ovin[s|v];
    PE transpose chunks -> psum_t[j, (c,p,h)]; copies assemble full
    output rows in out_asm; one fat 8KB-run DMA per half-group.
    """
    nc = bacc.Bacc("TRN2", target_bir_lowering=False)
    aw_d = nc.declare_dram_parameter("aw", [96, NPAIR, 128], BF16, isOutput=False)
    ar_d = nc.declare_dram_parameter("ar", [96, NPAIR, 64], BF16, isOutput=False)
    wg_d = nc.declare_dram_parameter("wg", [128, 8192], BF16, isOutput=False)
    id_d = nc.declare_dram_parameter("ident", [128, 128], BF16, isOutput=False)
    out_d = nc.declare_dram_parameter("out", [NL, A * G + A * H], F32, isOutput=True)

    Sq = mybir.ActivationFunctionType.Square
    ngroups = (NPAIR + GJ - 1) // GJ  # 5

    with tile.TileContext(nc) as tc:
        with (
            tc.tile_pool(name="singles", bufs=1) as singles,
            tc.tile_pool(name="lw", bufs=1) as lw_pool,
            tc.tile_pool(name="ar", bufs=1) as ar_pool,
            tc.tile_pool(name="vbig", bufs=2) as vbig_pool,
            tc.tile_pool(name="sq", bufs=2) as sq_pool,
            tc.tile_pool(name="ovin", bufs=16) as ovin_pool,
            tc.tile_pool(name="oasm", bufs=1) as oasm_pool,
            tc.tile_pool(name="psum1", bufs=2, space="PSUM") as p1_pool,
            tc.tile_pool(name="psum2", bufs=2, space="PSUM") as p2_pool,
            tc.tile_pool(name="psumt", bufs=2, space="PSUM") as pt_pool,
        ):
            wg = singles.tile([128, 8192], BF16)
            nc.sync.dma_start(out=wg[:, :], in_=wg_d[:, :])
            ident = singles.tile([128, 128], BF16)
            nc.sync.dma_start(out=ident[:, :], in_=id_d[:, :])

            # persistent rings: K-pad rows 96:128 must stay zero
            lws = [
                lw_pool.tile([128, KJ * 128], BF16, name=f"lw{i}") for i in range(3)
            ]
            ars = [
                ar_pool.tile([128, KJ * 64], BF16, name=f"arr{i}") for i in range(3)
            ]
            for lw in lws:
                nc.vector.memset(lw[96:128, :], 0.0)
            for arr in ars:
                nc.vector.memset(arr[96:128, :], 0.0)

            cglob = 0
            for g2 in range(ngroups):
                j0 = g2 * GJ
                jcnt = min(GJ, NPAIR - j0)  # pairs in group (256 or 226)
                n0 = 2 * j0
                vbig = vbig_pool.tile([128, A * GJ], BF16)
                vb = vbig[:, :].rearrange("p (a j) -> p a j", j=GJ)

                # ---- stage 1 ----
                nchunks = (jcnt + KJ - 1) // KJ
                for ck in range(nchunks):
                    cj0 = ck * KJ
                    cjc = min(KJ, jcnt - cj0)
                    Tg = j0 + cj0
                    lw = lws[cglob % 3]
                    arr = ars[cglob % 3]
                    cglob += 1
                    nc.sync.dma_start(
                        out=lw[0:96, 0 : 128 * cjc].rearrange(
                            "p (k z) -> p k z", z=128
                        ),
                        in_=aw_d[:, Tg : Tg + cjc, :],
                    )
                    nc.sync.dma_start(
                        out=arr[0:96, 0 : 64 * cjc].rearrange(
                            "p (k z) -> p k z", z=64
                        ),
                        in_=ar_d[:, Tg : Tg + cjc, :],
                    )
                    ntiles = (cjc + TP - 1) // TP
                    for t in range(ntiles):
                        tj0 = t * TP
                        tjc = min(TP, cjc - tj0)
                        psum1 = p1_pool.tile([128, 512], F32)
                        for k in range(tjc):
                            kk = tj0 + k
                            nc.tensor.matmul(
                                out=psum1[0:128, 64 * k : 64 * k + 64],
                                lhsT=lw[0:128, 128 * kk : 128 * kk + 128],
                                rhs=arr[0:128, 64 * kk : 64 * kk + 64],
                                start=True,
                                stop=True,
                            )
                        src = psum1[:, 0 : 64 * tjc].rearrange(
                            "p (k a) -> p a k", a=64
                        )
                        dst = vb[:, :, cj0 + tj0 : cj0 + tj0 + tjc]
                        if t % 2 == 0:
                            nc.vector.tensor_copy(out=dst, in_=src)
                        else:
                            nc.scalar.copy(out=dst, in_=src)

                # ---- stage 2 + finish part 1 ----
                ovins = []
                for q in range(16):
                    psum2 = p2_pool.tile([128, 1024], F32)
                    for s in range(4):
                        for c in range(4):
                            ch = 4 * q + c
                            c0 = 32 * (64 * s + ch)
                            nc.tensor.matmul(
                                out=psum2[32 * c : 32 * c + 32,
                                          256 * s : 256 * s + jcnt],
                                lhsT=wg[0:128, c0 : c0 + 32],
                                rhs=vb[0:128, ch, 0:jcnt],
                                start=True,
                                stop=True,
                                tile_position=(0, 32 * c),
                            )
                    sq = sq_pool.tile([128, 768], BF16)
                    nc.scalar.activation(
                        out=sq[:, :], in_=psum2[:, 0:768], func=Sq
                    )
                    ovin = ovin_pool.tile([128, 512], BF16)
                    ovins.append(ovin)
                    # ovin = [s-part 0:256 | v-part 256:512]
                    nc.vector.tensor_add(
                        ovin[:, 256:512], sq[:, 0:256], sq[:, 256:512]
                    )
                    nc.vector.tensor_add(
                        ovin[:, 256:512], ovin[:, 256:512], sq[:, 512:768]
                    )
                    nc.scalar.copy(out=ovin[:, 0:256], in_=psum2[:, 768:1024])

                # ---- finish part 2: transposes + assembly + out DMA ----
                jh_sizes = [min(128, jcnt), max(0, jcnt - 128)]
                asms = [None, None]
                for jh in range(2):
                    if jh_sizes[jh]:
                        asms[jh] = oasm_pool.tile(
                            [128, 4096], F32, name=f"asm{jh}"
                        )
                for q in range(16):
                    ovin = ovins[q]
                    psum_t = pt_pool.tile([128, 512], BF16)
                    for vs in range(2):
                        for jh in range(2):
                            jhc = jh_sizes[jh]
                            if not jhc:
                                continue
                            nc.tensor.transpose(
                                out=psum_t[0:jhc,
                                           128 * (2 * vs + jh) :
                                           128 * (2 * vs + jh) + 128],
                                in_=ovin[:, 256 * vs + 128 * jh :
                                         256 * vs + 128 * jh + jhc],
                                identity=ident[:, :],
                            )
                    # ident is a permutation: transposed labels come out (p,c,h)
                    ptv = psum_t[:, :].rearrange(
                        "z (v u p w) -> z v u p w", v=2, u=2, p=2
                    )
                    for jh in range(2):
                        jhc = jh_sizes[jh]
                        if not jhc:
                            continue
                        src = ptv[0:jhc, :, jh]
                        dst = asms[jh][0:jhc, :].rearrange(
                            "z (p v q w) -> z v p q w", p=2, v=2, q=16
                        )[:, :, :, q]
                        if q % 2 == 0:
                            nc.vector.tensor_copy(out=dst, in_=src)
                        else:
                            nc.scalar.copy(out=dst, in_=src)
                for jh in range(2):
                    jhc = jh_sizes[jh]
                    if not jhc:
                        continue
                    r0 = n0 + 256 * jh
                    nc.sync.dma_start(
                        out=out_d[r0 : r0 + 2 * jhc, :].rearrange(
                            "(j p) w -> j p w", p=2
                        ),
                        in_=asms[jh][0:jhc, :].rearrange(
                            "z (p w) -> z p w", p=2
                        ),
                    )
    nc.compile()
    return nc


def _get_nc():
    if "nc" not in _nc_cache:
        _nc_cache["nc"] = _build()
    return _nc_cache["nc"]


def _prep(a, gs, gv, agh):
    """Host-side packing into the per-core HBM layouts (bf16)."""
    a = np.asarray(a, np.float32)
    gs = np.asarray(gs, np.float32)
    gv = np.asarray(gv, np.float32)
    agh = np.asarray(agh, np.float32)

    # weights per atom: [gv d0 | gv d1 | gv d2 | gs] (16 each) -> 64 cols
    wcat = np.empty((N, M, 64), dtype=BF)
    for d in range(3):
        wcat[:, :, 16 * d : 16 * d + 16] = gv[:, :, :, d].astype(BF)
    wcat[:, :, 48:64] = gs.astype(BF)
    a16 = a.astype(BF)

    # aw[core][r=(x,m), j, 64x:64x+64] = wcat[n0+2j+x, m]; zeros elsewhere
    aw = np.zeros((NCORES, 96, NPAIR, 128), dtype=BF)
    ar = np.empty((NCORES, 96, NPAIR, 64), dtype=BF)
    wc = wcat.reshape(NCORES, NPAIR, 2, M, 64)
    ac = a16.reshape(NCORES, NPAIR, 2, M, 64)
    for x in range(2):
        aw[:, 48 * x : 48 * x + 48, :, 64 * x : 64 * x + 64] = wc[
            :, :, x
        ].transpose(0, 2, 1, 3)
        ar[:, 48 * x : 48 * x + 48] = ac[:, :, x].transpose(0, 2, 1, 3)

    # stage-2 weights: block (s, ch) = [128, 32], cols (p 2, h 16), value
    # agh[ch][g, h] (ident for s=3) at K-rows 64p+16s+g, zeros elsewhere
    wgm = np.zeros((128, 8192), dtype=BF)
    aghT = agh.transpose(1, 0, 2).astype(BF)  # [g, a, h]
    eye = np.eye(16, dtype=BF)
    for s in range(4):
        for ch in range(A):
            c0 = 32 * (64 * s + ch)
            blk = eye if s == 3 else aghT[:, ch, :]
            for p in range(2):
                r0 = 64 * p + 16 * s
                wgm[r0 : r0 + 16, c0 + 16 * p : c0 + 16 * p + 16] = blk

    # permutation for the PE transpose: label (c,p,h)=32c+16p+h goes to
    # output column (p,c,h)=64p+16c+h
    ident = np.zeros((128, 128), dtype=BF)
    for c in range(4):
        for p in range(2):
            for h in range(16):
                ident[32 * c + 16 * p + h, 64 * p + 16 * c + h] = 1
    return aw, ar, wgm, ident


def _in_maps(inputs):
    aw, ar, wgm, ident = _prep(
        inputs["a"], inputs["gs"], inputs["gv"], inputs["agh"]
    )
    return [
        {"aw": aw[c], "ar": ar[c], "wg": wgm, "ident": ident}
        for c in range(NCORES)
    ]


def kernel(a, gs, gv, agh):
    nc = _get_nc()
    in_maps = _in_maps({"a": a, "gs": gs, "gv": gv, "agh": agh})
    res = run_bass_kernel_spmd(nc, in_maps, list(range(NCORES))).results
    return np.concatenate([res[c]["out"] for c in range(NCORES)], axis=0)


# revision 40
# speedup vs baseline: 8.8307x; 1.1829x over previous
import os
import sys

sys.path.insert(0, "/opt/trn_rl_repo")

import numpy as np
import ml_dtypes

import concourse.bacc as bacc
import concourse.bass as bass
import concourse.mybir as mybir
import concourse.tile as tile
from concourse.bass_utils import run_bass_kernel_spmd

F32 = mybir.dt.float32
BF16 = mybir.dt.bfloat16
BF = ml_dtypes.bfloat16

N, M, G, A, H = 20000, 48, 16, 64, 16
NCORES = 8
NL = N // NCORES      # 2500 atoms per core
NPAIR = NL // 2       # 1250 atom pairs
GJ = 256              # pairs per group (512 atoms)
KJ = 32               # pairs per input DMA chunk
TP = 8                # pairs per psum1 tile

_nc_cache = {}


def _build():
    """Per-core Bass program, bf16 PE pipeline, fp32 psum/output.

    Stage 1 (per atom-pair): one [128,128]x[128,64] matmul.  lhsT rows
    (x=parity 0:96, zero rows 96:128 for the K=128 FWL pad), cols
    (x', dgs, g) with parity-zeros baked in HBM.  psum1 rows
    (x, dgs, g): e: d0@0,d1@16,d2@32,S'@48; o: +64.  Copy -> vbig bf16.
    Stage 2 (q-tile = 4 channels x 256 pairs): 16 matmuls, K=128 full
    rows, lhsT [128,32] (cols (p,h), baked zeros select slot+parity),
    column tiles (0,32c).  psum2 rows (c,p,h), cols (slot, j).
    Finish: ACT square -> sq bf16; DVE adds (d-sum) -> ovin[s|v];
    PE transpose chunks -> psum_t[j, (c,p,h)]; copies assemble full
    output rows in out_asm; one fat 8KB-run DMA per half-group.
    """
    nc = bacc.Bacc("TRN2", target_bir_lowering=False)
    aw_d = nc.declare_dram_parameter("aw", [96, NPAIR, 128], BF16, isOutput=False)
    ar_d = nc.declare_dram_parameter("ar", [96, NPAIR, 64], BF16, isOutput=False)
    wg_d = nc.declare_dram_parameter("wg", [128, 8192], BF16, isOutput=False)
    id_d = nc.declare_dram_parameter("ident", [128, 128], BF16, isOutput=False)
    out_d = nc.declare_dram_parameter("out", [NL, A * G + A * H], F32, isOutput=True)

    Sq = mybir.ActivationFunctionType.Square
    ngroups = (NPAIR + GJ - 1) // GJ  # 5

    with tile.TileContext(nc) as tc:
        with (
            tc.tile_pool(name="singles", bufs=1) as singles,
            tc.tile_pool(name="lw", bufs=1) as lw_pool,
            tc.tile_pool(name="ar", bufs=1) as ar_pool,
            tc.tile_pool(name="vbig", bufs=1) as vbig_pool,
            tc.tile_pool(name="sq", bufs=2) as sq_pool,
            tc.tile_pool(name="ovin", bufs=16) as ovin_pool,
            tc.tile_pool(name="oasm", bufs=1) as oasm_pool,
            tc.tile_pool(name="psum1", bufs=2, space="PSUM") as p1_pool,
            tc.tile_pool(name="psum2", bufs=2, space="PSUM") as p2_pool,
            tc.tile_pool(name="psumt", bufs=2, space="PSUM") as pt_pool,
        ):
            wg = singles.tile([128, 8192], BF16)
            nc.sync.dma_start(out=wg[:, :], in_=wg_d[:, :])
            ident = singles.tile([128, 128], BF16)
            nc.sync.dma_start(out=ident[:, :], in_=id_d[:, :])

            # persistent rings: K-pad rows 96:128 must stay zero
            lws = [
                lw_pool.tile([128, KJ * 128], BF16, name=f"lw{i}") for i in range(3)
            ]
            ars = [
                ar_pool.tile([128, KJ * 64], BF16, name=f"arr{i}") for i in range(3)
            ]
            for lw in lws:
                nc.gpsimd.memset(lw[96:128, :], 0.0)
            for arr in ars:
                nc.gpsimd.memset(arr[96:128, :], 0.0)

            state = {"cglob": 0}

            def stage1(g2):
                j0 = g2 * GJ
                jcnt = min(GJ, NPAIR - j0)  # pairs in group (256 or 226)
                vbig = vbig_pool.tile([128, A * GJ], BF16, name=f"vb{g2 % 2}")
                vb = vbig[:, :].rearrange("p (a j) -> p a j", j=GJ)

                nchunks = (jcnt + KJ - 1) // KJ
                for ck in range(nchunks):
                    cj0 = ck * KJ
                    cjc = min(KJ, jcnt - cj0)
                    Tg = j0 + cj0
                    lw = lws[state["cglob"] % 3]
                    arr = ars[state["cglob"] % 3]
                    state["cglob"] += 1
                    nc.sync.dma_start(
                        out=lw[0:96, 0 : 128 * cjc].rearrange(
                            "p (k z) -> p k z", z=128
                        ),
                        in_=aw_d[:, Tg : Tg + cjc, :],
                    )
                    nc.sync.dma_start(
                        out=arr[0:96, 0 : 64 * cjc].rearrange(
                            "p (k z) -> p k z", z=64
                        ),
                        in_=ar_d[:, Tg : Tg + cjc, :],
                    )
                    ntiles = (cjc + TP - 1) // TP
                    for t in range(ntiles):
                        tj0 = t * TP
                        tjc = min(TP, cjc - tj0)
                        psum1 = p1_pool.tile([128, 512], F32)
                        for k in range(tjc):
                            kk = tj0 + k
                            nc.tensor.matmul(
                                out=psum1[0:128, 64 * k : 64 * k + 64],
                                lhsT=lw[0:128, 128 * kk : 128 * kk + 128],
                                rhs=arr[0:128, 64 * kk : 64 * kk + 64],
                                start=True,
                                stop=True,
                            )
                        src = psum1[:, 0 : 64 * tjc].rearrange(
                            "p (k a) -> p a k", a=64
                        )
                        dst = vb[:, :, cj0 + tj0 : cj0 + tj0 + tjc]
                        if t % 2 == 0:
                            nc.vector.tensor_copy(out=dst, in_=src)
                        else:
                            nc.scalar.copy(out=dst, in_=src)
                return vbig

            def stage2_finish(g2, vbig):
                j0 = g2 * GJ
                jcnt = min(GJ, NPAIR - j0)
                n0 = 2 * j0
                vb = vbig[:, :].rearrange("p (a j) -> p a j", j=GJ)
                # ---- stage 2 + finish part 1 ----
                ovins = []
                for q in range(16):
                    psum2 = p2_pool.tile([128, 1024], F32)
                    for s in range(4):
                        for c in range(4):
                            ch = 4 * q + c
                            c0 = 32 * (64 * s + ch)
                            nc.tensor.matmul(
                                out=psum2[32 * c : 32 * c + 32,
                                          256 * s : 256 * s + jcnt],
                                lhsT=wg[0:128, c0 : c0 + 32],
                                rhs=vb[0:128, ch, 0:jcnt],
                                start=True,
                                stop=True,
                                tile_position=(0, 32 * c),
                            )
                    sq = sq_pool.tile([128, 768], BF16)
                    nc.scalar.activation(
                        out=sq[:, :], in_=psum2[:, 0:768], func=Sq
                    )
                    ovin = ovin_pool.tile([128, 512], BF16)
                    ovins.append(ovin)
                    # ovin = [s-part 0:256 | v-part 256:512]
                    nc.vector.tensor_add(
                        ovin[:, 256:512], sq[:, 0:256], sq[:, 256:512]
                    )
                    nc.vector.tensor_add(
                        ovin[:, 256:512], ovin[:, 256:512], sq[:, 512:768]
                    )
                    if q % 2 == 0:
                        nc.scalar.copy(out=ovin[:, 0:256], in_=psum2[:, 768:1024])
                    else:
                        nc.vector.tensor_copy(
                            out=ovin[:, 0:256], in_=psum2[:, 768:1024]
                        )

                # ---- finish part 2: transposes + assembly + out DMA ----
                jh_sizes = [min(128, jcnt), max(0, jcnt - 128)]
                asms = [None, None]
                for jh in range(2):
                    if jh_sizes[jh]:
                        asms[jh] = oasm_pool.tile(
                            [128, 4096], F32, name=f"asm{jh}"
                        )
                for q in range(16):
                    ovin = ovins[q]
                    psum_t = pt_pool.tile([128, 512], BF16)
                    for vs in range(2):
                        for jh in range(2):
                            jhc = jh_sizes[jh]
                            if not jhc:
                                continue
                            nc.tensor.transpose(
                                out=psum_t[0:jhc,
                                           128 * (2 * vs + jh) :
                                           128 * (2 * vs + jh) + 128],
                                in_=ovin[:, 256 * vs + 128 * jh :
                                         256 * vs + 128 * jh + jhc],
                                identity=ident[:, :],
                            )
                    # ident is a permutation: transposed labels come out (p,c,h)
                    ptv = psum_t[:, :].rearrange(
                        "z (v u p w) -> z v u p w", v=2, u=2, p=2
                    )
                    for jh in range(2):
                        jhc = jh_sizes[jh]
                        if not jhc:
                            continue
                        src = ptv[0:jhc, :, jh]
                        dst = asms[jh][0:jhc, :].rearrange(
                            "z (p v q w) -> z v p q w", p=2, v=2, q=16
                        )[:, :, :, q]
                        if q % 2 == 0:
                            nc.vector.tensor_copy(out=dst, in_=src)
                        else:
                            nc.scalar.copy(out=dst, in_=src)
                for jh in range(2):
                    jhc = jh_sizes[jh]
                    if not jhc:
                        continue
                    r0 = n0 + 256 * jh
                    nc.sync.dma_start(
                        out=out_d[r0 : r0 + 2 * jhc, :].rearrange(
                            "(j p) w -> j p w", p=2
                        ),
                        in_=asms[jh][0:jhc, :].rearrange(
                            "z (p w) -> z p w", p=2
                        ),
                    )

            # software pipeline: stage1(g) overlaps stage2+finish(g-1)
            prev = None
            for g2 in range(ngroups):
                vb_t = stage1(g2)
                if prev is not None:
                    stage2_finish(g2 - 1, prev)
                prev = vb_t
            stage2_finish(ngroups - 1, prev)
    nc.compile()
    return nc


def _get_nc():
    if "nc" not in _nc_cache:
        _nc_cache["nc"] = _build()
    return _nc_cache["nc"]


def _prep(a, gs, gv, agh):
    """Host-side packing into the per-core HBM layouts (bf16)."""
    a = np.asarray(a, np.float32)
    gs = np.asarray(gs, np.float32)
    gv = np.asarray(gv, np.float32)
    agh = np.asarray(agh, np.float32)

    # weights per atom: [gv d0 | gv d1 | gv d2 | gs] (16 each) -> 64 cols
    wcat = np.empty((N, M, 64), dtype=BF)
    for d in range(3):
        wcat[:, :, 16 * d : 16 * d + 16] = gv[:, :, :, d].astype(BF)
    wcat[:, :, 48:64] = gs.astype(BF)
    a16 = a.astype(BF)

    # aw[core][r=(x,m), j, 64x:64x+64] = wcat[n0+2j+x, m]; zeros elsewhere
    aw = np.zeros((NCORES, 96, NPAIR, 128), dtype=BF)
    ar = np.empty((NCORES, 96, NPAIR, 64), dtype=BF)
    wc = wcat.reshape(NCORES, NPAIR, 2, M, 64)
    ac = a16.reshape(NCORES, NPAIR, 2, M, 64)
    for x in range(2):
        aw[:, 48 * x : 48 * x + 48, :, 64 * x : 64 * x + 64] = wc[
            :, :, x
        ].transpose(0, 2, 1, 3)
        ar[:, 48 * x : 48 * x + 48] = ac[:, :, x].transpose(0, 2, 1, 3)

    # stage-2 weights: block (s, ch) = [128, 32], cols (p 2, h 16), value
    # agh[ch][g, h] (ident for s=3) at K-rows 64p+16s+g, zeros elsewhere
    wgm = np.zeros((128, 8192), dtype=BF)
    aghT = agh.transpose(1, 0, 2).astype(BF)  # [g, a, h]
    eye = np.eye(16, dtype=BF)
    for s in range(4):
        for ch in range(A):
            c0 = 32 * (64 * s + ch)
            blk = eye if s == 3 else aghT[:, ch, :]
            for p in range(2):
                r0 = 64 * p + 16 * s
                wgm[r0 : r0 + 16, c0 + 16 * p : c0 + 16 * p + 16] = blk

    # permutation for the PE transpose: label (c,p,h)=32c+16p+h goes to
    # output column (p,c,h)=64p+16c+h
    ident = np.zeros((128, 128), dtype=BF)
    for c in range(4):
        for p in range(2):
            for h in range(16):
                ident[32 * c + 16 * p + h, 64 * p + 16 * c + h] = 1
    return aw, ar, wgm, ident


def _in_maps(inputs):
    aw, ar, wgm, ident = _prep(
        inputs["a"], inputs["gs"], inputs["gv"], inputs["agh"]
    )
    return [
        {"aw": aw[c], "ar": ar[c], "wg": wgm, "ident": ident}
        for c in range(NCORES)
    ]


def kernel(a, gs, gv, agh):
    nc = _get_nc()
    in_maps = _in_maps({"a": a, "gs": gs, "gv": gv, "agh": agh})
    res = run_bass_kernel_spmd(nc, in_maps, list(range(NCORES))).results
    return np.concatenate([res[c]["out"] for c in range(NCORES)], axis=0)
